# revision 1
# baseline (speedup 1.0000x reference)
"""Trainium2 Bass kernel for nn_CrossSelfDecoder (B=4,N=1024,D=1024,H=16,F=4096).

Sharding: 8 cores = (batch b in 0..3) x (head-half hh in 0..1). Each core
computes attention for its 8 heads over all 1024 positions of its batch.
Because the reference reshapes (B,H,N,Dp)->(B,N,D) without permuting heads
back, head-ownership makes row-ownership invariant: core (b,hh) owns rows
[512*hh, 512*hh+512) of batch b through the whole network.

Activations are kept transposed ("T-domain": feature on partitions, row on
the free dim) so every GEMM contracts along partitions with no activation
transposes; only x1/x2 (inputs) and y (output) cross domains, via PE
transposes. Matmuls run as float32r (11-bit mantissa, full PE rate).
One pairwise AllGather exchanges the LN1 output so self-attention sees
keys/values from all positions.
"""

import os
import numpy as np

import concourse.mybir as mybir
import concourse.tile as tile
from concourse import bacc
from concourse.bass_utils import run_bass_kernel_spmd
from concourse.masks import make_identity

FP32 = mybir.dt.float32
FP32R = mybir.dt.float32r
AF = mybir.ActivationFunctionType
ALU = mybir.AluOpType

B, N, D, H, F = 4, 1024, 1024, 16, 4096
Dp = D // H           # 64
HPC = 8               # heads per core
PC = 128              # partition chunk
NF = 512              # free chunk (one psum bank of fp32)
KC = D // PC          # 8 contraction chunks
EPS = 1e-5

_CACHE = {}


def _round_fp32r(x):
    """Round-to-nearest-even onto fp32r's 1+8+11-bit grid (top 20 bits)."""
    x = np.ascontiguousarray(x, dtype=np.float32)
    b = x.view(np.uint32)
    low = b & np.uint32(0xFFF)
    half = np.uint32(0x800)
    bump = (low > half) | (
        (low == half) & (((b >> np.uint32(12)) & np.uint32(1)) != 0)
    )
    out = (b & np.uint32(0xFFFFF000)) + np.where(
        bump, np.uint32(0x1000), np.uint32(0)
    ).astype(np.uint32)
    return out.view(np.float32).copy()


def _r(ap):
    return ap.bitcast(FP32R)


def _build():
    nc = bacc.Bacc("TRN2", target_bir_lowering=False, debug=False,
                   num_devices=8)
    dram = {}
    for nm, shp in [
        ("x1", [N, D]), ("x2", [N, D]), ("x2r", [NF, D]),
        ("wq", [D, NF]), ("wk", [D, NF]), ("wv", [D, NF]),
        ("wq2", [D, NF]), ("wk2", [D, NF]), ("wv2", [D, NF]),
        ("w1", [D, F]), ("w2", [F, D]),
        ("bq", [NF]), ("bk", [NF]), ("bv", [NF]),
        ("bq2", [NF]), ("bk2", [NF]), ("bv2", [NF]),
        ("gamma", [D]), ("beta", [D]), ("b1", [F]), ("b2", [D]),
    ]:
        dram[nm] = nc.dram_tensor(nm, shp, FP32, kind="ExternalInput")
    y_out = nc.dram_tensor("y", [NF, D], FP32, kind="ExternalOutput")

    with tile.TileContext(nc) as tc:
        _emit(nc, tc, dram, y_out)
    nc.compile()
    return nc


def _emit(nc, tc, dram, y_out):
    with tc.tile_pool(name="persist", bufs=1) as pp:
        ident = pp.tile([PC, PC], FP32, tag="ident")
        make_identity(nc, ident[:])

        ones_sc = pp.tile([PC, 8], FP32, tag="ones_sc")
        nc.gpsimd.memset(ones_sc[:], 1.0)
        ones_row_raw = pp.tile([1, PC], FP32, tag="ones_row_raw")
        nc.gpsimd.memset(ones_row_raw[:], 1.0)
        eps_sc = pp.tile([1, 1], FP32, tag="eps_sc")
        nc.gpsimd.memset(eps_sc[:], EPS)
        # K=128,M=1 rounded ones column (LN sums lhsT)
        ones128 = pp.tile([PC, 1], FP32, tag="ones128")
        nc.scalar.copy(_r(ones128[:]), ones_sc[:, 0:1])
        # K=1,M=128 rounded ones row (broadcast lhsT)
        ones1 = pp.tile([1, PC], FP32, tag="ones1")
        nc.scalar.copy(_r(ones1[:]), ones_row_raw[:])

        def bias_cols(name, n):
            t = pp.tile([PC, n], FP32, tag=f"bc_{name}")
            nc.sync.dma_start(
                t[:], dram[name].ap().rearrange("(c p) -> p c", p=PC))
            return t

        bqT = bias_cols("bq", 4)
        bkT = bias_cols("bk", 4)
        bq2T = bias_cols("bq2", 4)
        bk2T = bias_cols("bk2", 4)
        gammaT = bias_cols("gamma", 8)
        betaT = bias_cols("beta", 8)
        b1T = bias_cols("b1", 32)
        b2T = bias_cols("b2", 8)

        bvR = pp.tile([1, NF], FP32, tag="bvR")
        nc.sync.dma_start(_r(bvR[:]), _r(dram["bv"].ap().unsqueeze(0)))
        bv2R = pp.tile([1, NF], FP32, tag="bv2R")
        nc.sync.dma_start(_r(bv2R[:]), _r(dram["bv2"].ap().unsqueeze(0)))

        consts = dict(
            ident=ident, ones_sc=ones_sc, ones128=ones128, ones1=ones1,
            bqT=bqT, bkT=bkT, bq2T=bq2T, bk2T=bk2T, gammaT=gammaT,
            betaT=betaT, b1T=b1T, b2T=b2T, bvR=bvR, bv2R=bv2R,
            eps_sc=eps_sc,
        )
        with tc.tile_pool(name="xdram", bufs=1, space="DRAM") as dp:
            # cross-stage DRAM: gather in/out and the LN2 output
            ag_in = dp.tile([N, NF], FP32, name="agin")
            ag_out = dp.tile([2 * N, NF], FP32, name="agout")
            n3d = dp.tile([D, NF], FP32, name="n3d")
            _stage1(nc, tc, dram, consts, ag_in, ag_out)
            _stage2(nc, tc, dram, consts, ag_in, ag_out, n3d)
            _stage3(nc, tc, dram, consts, n3d, y_out)


def _transpose_in(nc, tc, sub, ident, src_ap, nrows, dst_tiles, tag):
    """Transpose (nrows, 1024) DRAM -> 8 dst tiles (128, nrows):
    dst[c][p, row] = src[row, c*128+p]. Writes fp32r-rounded."""
    with tc.tile_pool(name=f"tp_{tag}", space="PSUM", bufs=1) as psp:
        for s in range(nrows // PC):
            strip = sub.tile([PC, D], FP32, tag="strip", bufs=3,
                             name=f"strip_{tag}{s}")
            nc.sync.dma_start(strip[:], src_ap[s * PC:(s + 1) * PC, :])
            for c in range(8):
                ps = psp.tile([PC, PC], FP32, tag="T", bufs=4,
                              name=f"tps_{tag}{s}_{c}")
                nc.tensor.transpose(ps[:], strip[:, c * PC:(c + 1) * PC],
                                    ident[:])
                nc.scalar.copy(
                    _r(dst_tiles[c][:, s * PC:(s + 1) * PC]), ps[:])


def _proj_T(nc, sub, psp, w_dram, bias_cols_tile, rhs_tiles, out_tiles, tag):
    """out[m] (128, 1024) = (W.T @ rhs + bias) in T-domain, m = 0..3."""
    for m in range(4):
        wt = sub.tile([PC, KC, PC], FP32, tag="w", bufs=4, name=f"w_{tag}{m}")
        nc.sync.dma_start(
            _r(wt[:]),
            _r(w_dram.ap()[:, m * PC:(m + 1) * PC]
               .rearrange("(kc p) f -> p kc f", p=PC)))
        for nf in range(2):
            ps = psp.tile([PC, NF], FP32, tag="proj", bufs=4,
                          name=f"proj_{tag}{m}_{nf}")
            for kc in range(KC):
                nc.tensor.matmul(
                    ps[:], _r(wt[:, kc, :]),
                    _r(rhs_tiles[kc][:, nf * NF:(nf + 1) * NF]),
                    start=(kc == 0), stop=(kc == KC - 1))
            nc.scalar.activation(
                _r(out_tiles[m][:, nf * NF:(nf + 1) * NF]), ps[:],
                AF.Identity, bias=bias_cols_tile[:, m:m + 1])


def _proj_v(nc, sub, psp, w_dram, bias_row, rhs_tiles, v_tiles, tag,
            ones_sc, ones1):
    """v natural (1024 x 512 own-head cols) + per-head ones column.
    v_tiles: 8 x (128, 520): head h data cols [65h,65h+64), col 65h+64=1."""
    wts = []
    for kc in range(KC):
        wt = sub.tile([PC, NF], FP32, tag="wv", bufs=8, name=f"wv_{tag}{kc}")
        nc.sync.dma_start(_r(wt[:]), _r(w_dram.ap()[kc * PC:(kc + 1) * PC, :]))
        wts.append(wt)
    bb = psp.tile([PC, NF], FP32, tag="aux", bufs=2, name=f"vb_{tag}")
    nc.tensor.matmul(bb[:], _r(ones1[:]), _r(bias_row[:]), start=True,
                     stop=True)
    bbs = sub.tile([PC, NF], FP32, tag="vbs", bufs=1, name=f"vbs_{tag}")
    nc.scalar.copy(bbs[:], bb[:])
    for pc in range(8):
        ps = psp.tile([PC, NF], FP32, tag="proj", bufs=4, name=f"v_{tag}{pc}")
        for kc in range(KC):
            nc.tensor.matmul(
                ps[:], _r(rhs_tiles[kc][:, pc * PC:(pc + 1) * PC]),
                _r(wts[kc][:]), start=(kc == 0), stop=(kc == KC - 1))
        vt = v_tiles[pc]
        vt3 = vt[:].rearrange("p (h c) -> p h c", h=HPC)
        ps3 = ps[:].rearrange("p (h c) -> p h c", h=HPC)
        nc.vector.tensor_tensor(
            _r(vt3[:, :, 0:Dp]), ps3[:], bbs[:].rearrange(
                "p (h c) -> p h c", h=HPC), op=ALU.add)
        nc.scalar.copy(_r(vt3[:, :, Dp:Dp + 1].squeeze(2)), ones_sc[:])


def _attention(nc, tc, sub, qT, kT, v_tiles, target_tiles, ones1, tag):
    """Own-head attention, scrambled-normalized write into target_tiles:
    target[j][64mm+d, 64*hloc+u] = O_norm[hloc][d, q=16u+(2j+mm)]."""
    with tc.tile_pool(name=f"attn_{tag}", space="PSUM", bufs=1) as psp:
        for hloc in range(HPC):
            t4, r64 = hloc // 2, Dp * (hloc % 2)
            for qh in range(2):
                pts = []
                for kc in range(KC):
                    sps = psp.tile([PC, NF], FP32, tag="S", bufs=3,
                                   name=f"S_{tag}{hloc}_{qh}_{kc}")
                    nc.tensor.matmul(
                        sps[:],
                        _r(kT[t4][r64:r64 + Dp, kc * PC:(kc + 1) * PC]),
                        _r(qT[t4][r64:r64 + Dp, qh * NF:(qh + 1) * NF]),
                        start=True, stop=True)
                    pt = sub.tile([PC, NF], FP32, tag="PT", bufs=10,
                                  name=f"PT_{tag}{hloc}_{qh}_{kc}")
                    nc.scalar.activation(_r(pt[:]), sps[:], AF.Exp)
                    pts.append(pt)
                ops = psp.tile([Dp + 1, NF], FP32, tag="O", bufs=2,
                               name=f"O_{tag}{hloc}_{qh}")
                for kc in range(KC):
                    nc.tensor.matmul(
                        ops[:], _r(v_tiles[kc][:, 65 * hloc:65 * hloc + 65]),
                        _r(pts[kc][:]), start=(kc == 0), stop=(kc == KC - 1))
                rrow = sub.tile([1, NF], FP32, tag="rrow", bufs=2,
                                name=f"rr_{tag}{hloc}_{qh}")
                nc.vector.reciprocal(rrow[:], ops[Dp:Dp + 1, :])
                rrowr = sub.tile([1, NF], FP32, tag="rrowr", bufs=2,
                                 name=f"rrr_{tag}{hloc}_{qh}")
                nc.scalar.copy(_r(rrowr[:]), rrow[:])
                rbp = psp.tile([Dp, NF], FP32, tag="aux", bufs=2,
                               name=f"rbp_{tag}{hloc}_{qh}")
                nc.tensor.matmul(rbp[:], _r(ones1[:, 0:Dp]), _r(rrowr[:]),
                                 start=True, stop=True)
                rb = sub.tile([Dp, NF], FP32, tag="rbs", bufs=2,
                              name=f"rb_{tag}{hloc}_{qh}")
                nc.scalar.copy(rb[:], rbp[:])
                for j in range(8):
                    for mm in range(2):
                        m = 2 * j + mm
                        src = ops[0:Dp, :].rearrange(
                            "d (u s) -> d s u", s=16)[:, m, :]
                        srb = rb[:].rearrange(
                            "d (u s) -> d s u", s=16)[:, m, :]
                        dst = target_tiles[j][
                            Dp * mm:Dp * mm + Dp,
                            Dp * hloc + 32 * qh:Dp * hloc + 32 * qh + 32]
                        nc.vector.tensor_tensor(_r(dst), src, srb, op=ALU.mult)


def _layernorm_T(nc, tc, sub, x_tiles, out_tiles, c, tag):
    """out[j] = LN(x) over the partition (feature) axis; out written fp32r."""
    ones128, ones1 = c["ones128"], c["ones1"]
    gammaT, betaT = c["gammaT"], c["betaT"]
    with tc.tile_pool(name=f"ln_{tag}", space="PSUM", bufs=1) as psp:
        s0 = psp.tile([1, NF], FP32, tag="s0", bufs=1, name=f"s0_{tag}")
        s1 = psp.tile([1, NF], FP32, tag="s1", bufs=1, name=f"s1_{tag}")
        for j in range(8):
            nc.tensor.matmul(s0[:], _r(ones128[:]), _r(x_tiles[j][:]),
                             start=(j == 0), stop=(j == 7))
            sq = sub.tile([PC, NF], FP32, tag="sq", bufs=2,
                          name=f"sq_{tag}{j}")
            nc.scalar.square(_r(sq[:]), x_tiles[j][:])
            nc.tensor.matmul(s1[:], _r(ones128[:]), _r(sq[:]),
                             start=(j == 0), stop=(j == 7))
        mu = sub.tile([1, NF], FP32, tag="lrow", bufs=4, name=f"mu_{tag}")
        nc.scalar.mul(mu[:], s0[:], 1.0 / D)
        msq = sub.tile([1, NF], FP32, tag="lrow", bufs=4, name=f"msq_{tag}")
        nc.scalar.mul(msq[:], s1[:], 1.0 / D)
        mu2 = sub.tile([1, NF], FP32, tag="lrow", bufs=4, name=f"mu2_{tag}")
        nc.scalar.square(mu2[:], mu[:])
        var = sub.tile([1, NF], FP32, tag="lrow", bufs=4, name=f"var_{tag}")
        nc.vector.tensor_sub(var[:], msq[:], mu2[:])
        std = sub.tile([1, NF], FP32, tag="lrow", bufs=4, name=f"std_{tag}")
        nc.scalar.activation(std[:], var[:], AF.Sqrt, bias=c["eps_sc"][:])
        rstd = sub.tile([1, NF], FP32, tag="lrow", bufs=4, name=f"rstd_{tag}")
        nc.vector.reciprocal(rstd[:], std[:])
        mur = sub.tile([1, NF], FP32, tag="lrow", bufs=4, name=f"mur_{tag}")
        nc.scalar.copy(_r(mur[:]), mu[:])
        rstdr = sub.tile([1, NF], FP32, tag="lrow", bufs=4,
                         name=f"rstdr_{tag}")
        nc.scalar.copy(_r(rstdr[:]), rstd[:])
        mub = sub.tile([PC, NF], FP32, tag="lnb", bufs=2, name=f"mub_{tag}")
        bb = psp.tile([PC, NF], FP32, tag="lnbc", bufs=1, name=f"mubp_{tag}")
        nc.tensor.matmul(bb[:], _r(ones1[:]), _r(mur[:]), start=True,
                         stop=True)
        nc.scalar.copy(mub[:], bb[:])
        rstdb = sub.tile([PC, NF], FP32, tag="lnb", bufs=2, name=f"rsb_{tag}")
        bb2 = psp.tile([PC, NF], FP32, tag="lnbc", bufs=1, name=f"rsbp_{tag}")
        nc.tensor.matmul(bb2[:], _r(ones1[:]), _r(rstdr[:]), start=True,
                         stop=True)
        nc.scalar.copy(rstdb[:], bb2[:])
        for j in range(8):
            t1 = sub.tile([PC, NF], FP32, tag="lntmp", bufs=2,
                          name=f"lt_{tag}{j}")
            nc.vector.tensor_sub(t1[:], x_tiles[j][:], mub[:])
            nc.vector.tensor_mul(t1[:], t1[:], rstdb[:])
            nc.scalar.activation(
                _r(out_tiles[j]), t1[:], AF.Identity,
                bias=betaT[:, j:j + 1], scale=gammaT[:, j:j + 1])


def _stage1(nc, tc, dram, c, ag_in, ag_out):
    with tc.tile_pool(name="s1", bufs=1) as s1:
        x2ownT = [s1.tile([PC, NF], FP32, tag="x2ownT", bufs=8,
                          name=f"x2ownT{i}") for i in range(8)]
        qT = [s1.tile([PC, N], FP32, tag="qT", bufs=4, name=f"qT{i}")
              for i in range(4)]
        kT = [s1.tile([PC, N], FP32, tag="kT", bufs=4, name=f"kT{i}")
              for i in range(4)]
        v_tiles = [s1.tile([PC, 65 * HPC], FP32, tag="v", bufs=8,
                           name=f"v{i}") for i in range(8)]
        xT = [s1.tile([PC, NF], FP32, tag="xT", bufs=8, name=f"xT{i}")
              for i in range(8)]

        # phase A: transposes of x2 (full) and x2r (own rows); q projection
        with tc.tile_pool(name="s1a", bufs=1) as sub:
            x2T = [sub.tile([PC, N], FP32, tag="x2T", bufs=8, name=f"x2T{i}")
                   for i in range(8)]
            _transpose_in(nc, tc, sub, c["ident"], dram["x2"].ap(), N,
                          x2T, "x2")
            _transpose_in(nc, tc, sub, c["ident"], dram["x2r"].ap(), NF,
                          x2ownT, "x2r")
            with tc.tile_pool(name="s1ap", space="PSUM", bufs=1) as psp:
                _proj_T(nc, sub, psp, dram["wq"], c["bqT"], x2T, qT, "q")

        # phase B: x1 transpose; k,v projections
        with tc.tile_pool(name="s1b", bufs=1) as sub:
            x1T = [sub.tile([PC, N], FP32, tag="x1T", bufs=8, name=f"x1T{i}")
                   for i in range(8)]
            _transpose_in(nc, tc, sub, c["ident"], dram["x1"].ap(), N,
                          x1T, "x1")
            with tc.tile_pool(name="s1bp", space="PSUM", bufs=1) as psp:
                _proj_T(nc, sub, psp, dram["wk"], c["bkT"], x1T, kT, "k")
                _proj_v(nc, sub, psp, dram["wv"], c["bvR"], x1T, v_tiles,
                        "v1", c["ones_sc"], c["ones1"])

        # phase C: attention + residual + LN1 + all-gather
        with tc.tile_pool(name="s1c", bufs=1) as sub:
            _attention(nc, tc, sub, qT, kT, v_tiles, xT, c["ones1"], "x")
            for j in range(8):
                nc.vector.tensor_tensor(_r(xT[j][:]), xT[j][:],
                                        x2ownT[j][:], op=ALU.add)
            nTo = [sub.tile([PC, NF], FP32, tag="nTo", bufs=8,
                            name=f"nTo{i}") for i in range(8)]
            _layernorm_T(nc, tc, sub, xT, [t[:] for t in nTo], c, "ln1")
            for j in range(8):
                nc.sync.dma_start(ag_in[j * PC:(j + 1) * PC, :], nTo[j][:])
            if os.environ.get("KBENCH_NO_CC", "0") == "1":
                # timing stand-in for TimelineSim (no collectives there)
                nc.sync.dma_start(ag_out[0:N, :], ag_in[:])
                nc.sync.dma_start(ag_out[N:2 * N, :], ag_in[:])
            else:
                nc.gpsimd.collective_compute(
                    "AllGather", ALU.bypass,
                    replica_groups=[[0, 1], [2, 3], [4, 5], [6, 7]],
                    ins=[ag_in[:]], outs=[ag_out[:]])


def _stage2(nc, tc, dram, c, ag_in, ag_out, n3d):
    with tc.tile_pool(name="s2", bufs=1) as s2:
        nT_full = [s2.tile([PC, N], FP32, tag="nTf", bufs=8, name=f"nTf{i}")
                   for i in range(8)]
        nTo2 = [s2.tile([PC, NF], FP32, tag="nTo2", bufs=8, name=f"nTo2_{i}")
                for i in range(8)]
        gsrc = ag_out[:].rearrange("(h q) cc -> h q cc", h=2)
        for j in range(8):
            nc.sync.dma_start(
                _r(nT_full[j][:].rearrange("p (h cc) -> p h cc", h=2)),
                _r(gsrc[:, j * PC:(j + 1) * PC, :].transpose([1, 0, 2])))
            nc.sync.dma_start(nTo2[j][:], ag_in[j * PC:(j + 1) * PC, :])
        qT = [s2.tile([PC, N], FP32, tag="q2T", bufs=4, name=f"q2T{i}")
              for i in range(4)]
        kT = [s2.tile([PC, N], FP32, tag="k2T", bufs=4, name=f"k2T{i}")
              for i in range(4)]
        v_tiles = [s2.tile([PC, 65 * HPC], FP32, tag="v2", bufs=8,
                           name=f"v2_{i}") for i in range(8)]
        xT = [s2.tile([PC, NF], FP32, tag="x3T", bufs=8, name=f"x3T{i}")
              for i in range(8)]
        with tc.tile_pool(name="s2a", bufs=1) as sub:
            with tc.tile_pool(name="s2ap", space="PSUM", bufs=1) as psp:
                _proj_T(nc, sub, psp, dram["wq2"], c["bq2T"], nT_full, qT,
                        "q2")
                _proj_T(nc, sub, psp, dram["wk2"], c["bk2T"], nT_full, kT,
                        "k2")
                _proj_v(nc, sub, psp, dram["wv2"], c["bv2R"], nT_full,
                        v_tiles, "v2", c["ones_sc"], c["ones1"])
        with tc.tile_pool(name="s2b", bufs=1) as sub:
            _attention(nc, tc, sub, qT, kT, v_tiles, xT, c["ones1"], "y")
            for j in range(8):
                nc.vector.tensor_tensor(_r(xT[j][:]), xT[j][:], nTo2[j][:],
                                        op=ALU.add)
            n3T = [sub.tile([PC, NF], FP32, tag="n3T", bufs=8,
                            name=f"n3T{i}") for i in range(8)]
            _layernorm_T(nc, tc, sub, xT, [t[:] for t in n3T], c, "ln2")
            for j in range(8):
                nc.sync.dma_start(n3d[j * PC:(j + 1) * PC, :], n3T[j][:])


def _stage3(nc, tc, dram, c, n3d, y_out):
    FC = F // PC  # 32
    with tc.tile_pool(name="s3", bufs=1) as s3:
        n3T = [s3.tile([PC, NF], FP32, tag="n3T", bufs=8, name=f"n3Tb{i}")
               for i in range(8)]
        for j in range(8):
            nc.sync.dma_start(_r(n3T[j][:]), _r(n3d[j * PC:(j + 1) * PC, :]))
        hT = [s3.tile([PC, NF], FP32, tag="hT", bufs=FC, name=f"hT{i}")
              for i in range(FC)]
        with tc.tile_pool(name="s3p", space="PSUM", bufs=1) as psp:
            for f in range(FC):
                wt = s3.tile([PC, KC, PC], FP32, tag="w1t", bufs=4,
                             name=f"w1t{f}")
                nc.sync.dma_start(
                    _r(wt[:]),
                    _r(dram["w1"].ap()[:, f * PC:(f + 1) * PC]
                       .rearrange("(kc p) ff -> p kc ff", p=PC)))
                ps = psp.tile([PC, NF], FP32, tag="proj", bufs=4,
                              name=f"h{f}")
                for kc in range(KC):
                    nc.tensor.matmul(ps[:], _r(wt[:, kc, :]), _r(n3T[kc][:]),
                                     start=(kc == 0), stop=(kc == KC - 1))
                nc.scalar.activation(_r(hT[f][:]), ps[:], AF.Gelu,
                                     bias=c["b1T"][:, f:f + 1])
            yT = [s3.tile([PC, NF], FP32, tag="yT", bufs=8, name=f"yT{i}")
                  for i in range(8)]
            for d in range(8):
                w2t = []
                for half in range(2):
                    t = s3.tile([PC, 16, PC], FP32, tag="w2t", bufs=2,
                                name=f"w2t{d}_{half}")
                    nc.sync.dma_start(
                        _r(t[:]),
                        _r(dram["w2"].ap()[half * 2048:(half + 1) * 2048,
                                           d * PC:(d + 1) * PC]
                           .rearrange("(fc p) dd -> p fc dd", p=PC)))
                    w2t.append(t)
                ps = psp.tile([PC, NF], FP32, tag="proj", bufs=4,
                              name=f"yp{d}")
                for f in range(FC):
                    nc.tensor.matmul(
                        ps[:], _r(w2t[f // 16][:, f % 16, :]), _r(hT[f][:]),
                        start=(f == 0), stop=(f == FC - 1))
                nc.vector.scalar_tensor_tensor(
                    yT[d][:], ps[:], c["b2T"][:, d:d + 1], n3T[d][:],
                    op0=ALU.add, op1=ALU.add)
            for t in range(4):
                for d in range(8):
                    ps = psp.tile([PC, PC], FP32, tag="yt", bufs=4,
                                  name=f"ytp{t}_{d}")
                    nc.tensor.transpose(
                        ps[:], yT[d][:, t * PC:(t + 1) * PC], c["ident"][:])
                    yn = s3.tile([PC, PC], FP32, tag="yn", bufs=4,
                                 name=f"yn{t}_{d}")
                    nc.scalar.copy(yn[:], ps[:])
                    nc.sync.dma_start(
                        y_out.ap()[t * PC:(t + 1) * PC,
                                   d * PC:(d + 1) * PC], yn[:])


def _get_nc():
    if "nc" not in _CACHE:
        _CACHE["nc"] = _build()
    return _CACHE["nc"]


def kernel(**inputs):
    x1 = np.ascontiguousarray(np.asarray(inputs["x1"], np.float32))
    x2 = np.ascontiguousarray(np.asarray(inputs["x2"], np.float32))
    Wq = _round_fp32r(inputs["Wq"])
    Wkv = _round_fp32r(inputs["Wkv"])
    Wqkv = _round_fp32r(inputs["Wqkv"])
    W1 = _round_fp32r(inputs["W1"])
    W2 = _round_fp32r(inputs["W2"])
    bq = _round_fp32r(inputs["bq"])
    bkv = _round_fp32r(inputs["bkv"])
    bqkv = _round_fp32r(inputs["bqkv"])
    gamma = np.ascontiguousarray(np.asarray(inputs["gamma"], np.float32))
    beta = np.ascontiguousarray(np.asarray(inputs["beta"], np.float32))
    b1 = np.ascontiguousarray(np.asarray(inputs["b1"], np.float32))
    b2 = np.ascontiguousarray(np.asarray(inputs["b2"], np.float32))

    nc = _get_nc()
    in_maps = []
    for core in range(8):
        b, hh = core // 2, core % 2
        lo = NF * hh
        in_maps.append({
            "x1": x1[b], "x2": x2[b],
            "x2r": np.ascontiguousarray(x2[b, lo:lo + NF, :]),
            "wq": np.ascontiguousarray(Wq[:, lo:lo + NF]),
            "wk": np.ascontiguousarray(Wkv[:, lo:lo + NF]),
            "wv": np.ascontiguousarray(Wkv[:, D + lo:D + lo + NF]),
            "wq2": np.ascontiguousarray(Wqkv[:, lo:lo + NF]),
            "wk2": np.ascontiguousarray(Wqkv[:, D + lo:D + lo + NF]),
            "wv2": np.ascontiguousarray(Wqkv[:, 2 * D + lo:2 * D + lo + NF]),
            "w1": W1, "w2": W2,
            "bq": np.ascontiguousarray(bq[lo:lo + NF]),
            "bk": np.ascontiguousarray(bkv[lo:lo + NF]),
            "bv": np.ascontiguousarray(bkv[D + lo:D + lo + NF]),
            "bq2": np.ascontiguousarray(bqkv[lo:lo + NF]),
            "bk2": np.ascontiguousarray(bqkv[D + lo:D + lo + NF]),
            "bv2": np.ascontiguousarray(bqkv[2 * D + lo:2 * D + lo + NF]),
            "gamma": gamma, "beta": beta, "b1": b1, "b2": b2,
        })
    res = run_bass_kernel_spmd(nc, in_maps, core_ids=list(range(8)))
    _CACHE["last_results"] = res
    out = np.zeros((B, N, D), np.float32)
    for core in range(8):
        b, hh = core // 2, core % 2
        out[b, NF * hh:NF * hh + NF, :] = res.results[core]["y"]
    return out



# revision 22
# speedup vs baseline: 1.2529x; 1.2529x over previous
"""Trainium2 Bass kernel for nn_CrossSelfDecoder (B=4,N=1024,D=1024,H=16,F=4096).

Sharding: 8 cores = (batch b in 0..3) x (head-half hh in 0..1). Each core
computes attention for its 8 heads over all 1024 positions of its batch.
Because the reference reshapes (B,H,N,Dp)->(B,N,D) without permuting heads
back, head-ownership makes row-ownership invariant: core (b,hh) owns rows
[512*hh, 512*hh+512) of batch b through the whole network.

v2 layout strategy vs v1:
- x1/x2 are pre-transposed on the HOST (numpy) and uploaded as [D, N], so
  no PE transposes / PSUM->SBUF copies are needed on-chip. The output is
  produced transposed [D, NF] and transposed back on the host.
- Weights are pre-arranged on the host into the exact SBUF tile layouts so
  every weight DMA is a contiguous [128, X] block read.
- Softmax exp runs on 2-bank PSUM tiles [128, 1024] (both query halves per
  (head, key-block)), halving ACT instruction count.
- Softmax P and V tiles are bf16 (pure-bf16 PV matmuls, full PE rate).
- Attention normalize uses reciprocal_approx_fast + two merged [64, 8x32]
  strided DVE multiplies per (head, qh) instead of 16 tiny ops.
- Projection bias-adds moved to DVE (tensor_scalar_add).
- LN1 output round-trips through the AllGather in natural row order (rank
  order [even, odd] == row order), so one compiled program serves all
  cores; LN2 output stays in SBUF (no DRAM round trip).
"""

import os
import numpy as np

import concourse.mybir as mybir
import concourse.tile as tile
from concourse import bacc
from concourse.bass_utils import run_bass_kernel_spmd

FP32 = mybir.dt.float32
FP32R = mybir.dt.float32r
BF16 = mybir.dt.bfloat16
AF = mybir.ActivationFunctionType
ALU = mybir.AluOpType

B, N, D, H, F = 4, 1024, 1024, 16, 4096
Dp = D // H           # 64
HPC = 8               # heads per core
PC = 128              # partition chunk
NF = 512              # free chunk (one psum bank of fp32)
KC = D // PC          # 8 contraction chunks
FC = F // PC          # 32
EPS = 1e-5

_CACHE = {}


def _round_fp32r(x):
    """Round-to-nearest-even onto fp32r's 1+8+11-bit grid (top 20 bits)."""
    x = np.ascontiguousarray(x, dtype=np.float32)
    b = x.view(np.uint32)
    low = b & np.uint32(0xFFF)
    half = np.uint32(0x800)
    bump = (low > half) | (
        (low == half) & (((b >> np.uint32(12)) & np.uint32(1)) != 0)
    )
    out = (b & np.uint32(0xFFFFF000)) + np.where(
        bump, np.uint32(0x1000), np.uint32(0)
    ).astype(np.uint32)
    return out.view(np.float32).copy()


def _r(ap):
    return ap.bitcast(FP32R)


KDEBUG = os.environ.get("KDEBUG", "0") == "1"
_dbg_dram = {}


def _build():
    global _dbg_dram
    nc = bacc.Bacc("TRN2", target_bir_lowering=False, debug=False,
                   num_devices=8)
    dram = {}
    _dbg_dram = dram
    if KDEBUG:
        for nm, shp in [
            ("dbg_qT", [4 * PC, N]), ("dbg_kT", [4 * PC, N]),
            ("dbg_x1a", [PC, 8 * NF]), ("dbg_nTo", [N, NF]),
            ("dbg_nT", [N, N]), ("dbg_x3a", [PC, 8 * NF]),
            ("dbg_n3", [N, NF]), ("dbg_v", [N, NF]),
            ("dbg_pt", [PC, N]), ("dbg_ops", [PC, NF]),
        ]:
            dram[nm] = nc.dram_tensor(nm, shp, FP32, kind="ExternalOutput")
    for nm, shp in [
        ("x1t", [D, N]), ("x2t", [D, N]), ("x2o", [D, NF]),
        # wq_t/wk_t: [4*128, 8*128]; row m*128+p, col kc*128+c =
        #   W[kc*128+p, m*128+c] (own 512-col slice)
        ("wqt", [4 * PC, D]), ("wkt", [4 * PC, D]),
        ("wq2t", [4 * PC, D]), ("wk2t", [4 * PC, D]),
        # wv: natural [D, 512] own-col slice
        ("wv", [D, NF]), ("wv2", [D, NF]),
        # w1_t: [32*128, 8*128]; row f*128+p, col kc*128+c = W1[kc*128+p, f*128+c]
        ("w1t", [F, D]),
        # w2_t: [8*128, 32*128]; row d*128+p, col fc*128+c = W2[fc*128+p, d*128+c]
        ("w2t", [D, F]),
        ("bqc", [PC, 4]), ("bkc", [PC, 4]),
        ("bq2c", [PC, 4]), ("bk2c", [PC, 4]),
        ("bv", [NF]), ("bv2", [NF]),
        ("gammac", [PC, 8]), ("betac", [PC, 8]),
        ("b1c", [PC, FC]), ("b2c", [PC, 8]),
    ]:
        dram[nm] = nc.dram_tensor(nm, shp, FP32, kind="ExternalInput")
    y_out = nc.dram_tensor("y", [D, NF], FP32, kind="ExternalOutput")

    with tile.TileContext(nc) as tc:
        _emit(nc, tc, dram, y_out)
    nc.compile()
    return nc


def _proj_qk(nc, tc, pool, w_dram, bias_tile, rhs_fn, out_tiles, tag):
    """T-domain projection: out[m] [128, 1024] fp32r = W.T @ x + bias.

    kc-outer loop so the rhs activation chunks can be streamed.
    rhs_fn(kc) -> [128, 1024] AP (feature chunk kc, all 1024 rows).
    """
    with tc.tile_pool(name=f"proj_{tag}", space="PSUM", bufs=1) as psp:
        wts, pss = [], []
        for m in range(4):
            wt = pool.tile([PC, D], FP32, tag="wqk", bufs=4,
                           name=f"w_{tag}{m}")
            nc.sync.dma_start(_r(wt[:]),
                              _r(w_dram.ap()[m * PC:(m + 1) * PC, :]))
            wts.append(wt)
            pss.append(psp.tile([PC, N], FP32, tag="proj", bufs=4,
                                name=f"p_{tag}{m}"))
        for kc in range(KC):
            rhs = rhs_fn(kc)
            for m in range(4):
                w = _r(wts[m][:, kc * PC:(kc + 1) * PC])
                nc.tensor.matmul(pss[m][:, 0:NF], w, _r(rhs[:, 0:NF]),
                                 start=(kc == 0), stop=(kc == KC - 1))
                nc.tensor.matmul(pss[m][:, NF:N], w, _r(rhs[:, NF:N]),
                                 start=(kc == 0), stop=(kc == KC - 1))
        for m in range(4):
            nc.vector.tensor_scalar_add(_r(out_tiles[m][:]), pss[m][:],
                                        bias_tile[:, m:m + 1])


def _proj_v(nc, tc, pool, w_dram, bias_row_dram, lhsT_fn, v_tiles, tag,
            ones_sc, ones1):
    """v natural (1024 rows x 512 own-head cols) + per-head ones column.
    v_tiles: 8 x (128, 520) bf16: head h cols [65h,65h+64), col 65h+64=1.
    lhsT_fn(kc) -> [128, 1024] AP (feature chunk kc on partitions, rows on
    free); slices of it are the per-pc stationary operands. Bias is folded
    into PSUM as a K=1 broadcast matmul."""
    bvR = pool.tile([1, NF], FP32, tag="bvr", bufs=2, name=f"bvR_{tag}")
    nc.sync.dma_start(_r(bvR[:]), _r(bias_row_dram.ap().unsqueeze(0)))
    with tc.tile_pool(name=f"projv_{tag}", space="PSUM", bufs=1) as psp:
        pss = [psp.tile([PC, NF], FP32, tag="vproj", bufs=8,
                        name=f"v_{tag}{pc}") for pc in range(8)]
        for pc in range(8):
            nc.tensor.matmul(pss[pc][:], _r(ones1[:]), _r(bvR[:]),
                             start=True, stop=False)
        for kc in range(KC):
            wt = pool.tile([PC, NF], FP32, tag="wvs", bufs=3,
                           name=f"wv_{tag}{kc}")
            nc.sync.dma_start(_r(wt[:]),
                              _r(w_dram.ap()[kc * PC:(kc + 1) * PC, :]))
            lhsT = lhsT_fn(kc)
            for pc in range(8):
                nc.tensor.matmul(pss[pc][:],
                                 _r(lhsT[:, pc * PC:(pc + 1) * PC]),
                                 _r(wt[:]), start=False, stop=(kc == KC - 1))
        for pc in range(8):
            # head block: col 96h = ones (denominator -> PSUM partition 0),
            # cols 96h+32..96h+96 = data (PSUM partitions 32..96, aligned)
            vt3 = v_tiles[pc][:].rearrange("p (h c) -> p h c", h=HPC)
            nc.vector.tensor_copy(
                vt3[:, :, 64:128],
                pss[pc][:].rearrange("p (h c) -> p h c", h=HPC))
            nc.scalar.copy(vt3[:, :, 0:1].squeeze(2), ones_sc[:])


def _attention(nc, tc, pool, qT, kT, v_tiles, xT_all, ones1, tag):
    """Own-head attention, normalized + scrambled into xT_all [128, 4096].

    xT_all[64*mm + d, 512*j + 64*hloc + 32*qh + u] =
        O_norm[hloc][d, q = 512*qh + 16*u + 2*j + mm]
    """
    with tc.tile_pool(name=f"attn_{tag}", space="PSUM", bufs=1) as psp:
        for hloc in range(HPC):
            t4, r64 = hloc // 2, Dp * (hloc % 2)
            opss = [psp.tile([PC, NF], FP32, tag="O", bufs=3,
                             name=f"O_{tag}{hloc}_{qh}") for qh in range(2)]
            for kc in range(KC):
                sps = psp.tile([PC, N], FP32, tag="S", bufs=2,
                               name=f"S_{tag}{hloc}_{kc}")
                for qh in range(2):
                    nc.tensor.matmul(
                        sps[:, qh * NF:(qh + 1) * NF],
                        _r(kT[t4][r64:r64 + Dp, kc * PC:(kc + 1) * PC]),
                        _r(qT[t4][r64:r64 + Dp, qh * NF:(qh + 1) * NF]),
                        start=True, stop=True)
                pt = pool.tile([PC, N], BF16, tag="PT", bufs=3,
                               name=f"PT_{tag}{hloc}_{kc}")
                nc.scalar.activation(pt[:], sps[:], AF.Exp)
                if KDEBUG and tag == "x" and hloc == 0 and kc == 0:
                    ptf = pool.tile([PC, N], FP32, tag="ptdbg", bufs=1,
                                    name="ptdbg")
                    nc.vector.tensor_copy(ptf[:], pt[:])
                    nc.sync.dma_start(
                        _dbg_dram["dbg_pt"].ap(), ptf[:])
                for qh in range(2):
                    nc.tensor.matmul(
                        opss[qh][:],
                        v_tiles[kc][:, 128 * hloc:128 * hloc + 128],
                        pt[:, qh * NF:(qh + 1) * NF],
                        start=(kc == 0), stop=(kc == KC - 1))
            if KDEBUG and tag == "x" and hloc == 0:
                of = pool.tile([PC, NF], FP32, tag="opsdbg", bufs=1,
                               name="opsdbg")
                nc.vector.tensor_copy(of[:], opss[0][:])
                nc.sync.dma_start(_dbg_dram["dbg_ops"].ap(), of[:])
            for qh in range(2):
                ops = opss[qh]
                # row 0 = denominator (ones col first); rows 1..64 = data.
                # reciprocal_approx_fast (custom DVE) requires its input at
                # partition offset 0 — the offset is dropped otherwise.
                rrow = pool.tile([1, NF], FP32, tag="rrow", bufs=2,
                                 name=f"rr_{tag}{hloc}_{qh}")
                nc.vector.reciprocal_approx_fast(rrow[:], ops[0:1, :])
                rrowr = pool.tile([1, NF], FP32, tag="rrowr", bufs=2,
                                  name=f"rrr_{tag}{hloc}_{qh}")
                nc.vector.tensor_copy(_r(rrowr[:]), rrow[:])
                rbp = psp.tile([Dp, NF], FP32, tag="rb", bufs=1,
                               name=f"rbp_{tag}{hloc}_{qh}")
                nc.tensor.matmul(rbp[:], _r(ones1[:, 0:Dp]), _r(rrowr[:]),
                                 start=True, stop=True)
                rb = pool.tile([Dp, NF], FP32, tag="rbs", bufs=2,
                               name=f"rb_{tag}{hloc}_{qh}")
                nc.vector.tensor_copy(rb[:], rbp[:])
                # merged scramble-normalize: 2 ops of [64, (j:8, u:32)]
                src4 = ops[64:128, :].rearrange(
                    "d (u j2 m2) -> d j2 u m2", u=32, j2=8)
                rb4 = rb[:].rearrange("d (u j2 m2) -> d j2 u m2", u=32, j2=8)
                for mm in range(2):
                    dst = xT_all[Dp * mm:Dp * mm + Dp, :].rearrange(
                        "d (j r) -> d j r", j=8)[
                        :, :, Dp * hloc + 32 * qh:Dp * hloc + 32 * qh + 32]
                    nc.vector.tensor_tensor(
                        _r(dst), src4[:, :, :, mm], rb4[:, :, :, mm],
                        op=ALU.mult)


def _layernorm_T(nc, tc, pool, x_all, out_tiles, c, tag):
    """out[j] [128, 512] fp32r = LN(x_all [128, 4096]) over features
    (partitions x 8 chunks)."""
    ones128, ones1 = c["ones128"], c["ones1"]
    gammaT, betaT = c["gammaT"], c["betaT"]
    with tc.tile_pool(name=f"ln_{tag}", space="PSUM", bufs=1) as psp:
        s0 = psp.tile([1, NF], FP32, tag="s0", bufs=1, name=f"s0_{tag}")
        s1 = psp.tile([1, NF], FP32, tag="s1", bufs=1, name=f"s1_{tag}")
        for j in range(8):
            xj = x_all[:, j * NF:(j + 1) * NF]
            nc.tensor.matmul(s0[:], _r(ones128[:]), _r(xj), start=(j == 0),
                             stop=(j == 7))
            sq = pool.tile([PC, NF], FP32, tag="sq", bufs=2,
                           name=f"sq_{tag}{j}")
            nc.vector.tensor_tensor(_r(sq[:]), xj, xj, op=ALU.mult)
            nc.tensor.matmul(s1[:], _r(ones128[:]), _r(sq[:]),
                             start=(j == 0), stop=(j == 7))
        mu = pool.tile([1, NF], FP32, tag="lrow", bufs=6, name=f"mu_{tag}")
        nc.scalar.mul(mu[:], s0[:], 1.0 / D)
        msq = pool.tile([1, NF], FP32, tag="lrow", bufs=6, name=f"msq_{tag}")
        nc.scalar.mul(msq[:], s1[:], 1.0 / D)
        mu2 = pool.tile([1, NF], FP32, tag="lrow", bufs=6, name=f"mu2_{tag}")
        nc.scalar.square(mu2[:], mu[:])
        var = pool.tile([1, NF], FP32, tag="lrow", bufs=6, name=f"var_{tag}")
        nc.vector.tensor_sub(var[:], msq[:], mu2[:])
        std = pool.tile([1, NF], FP32, tag="lrow", bufs=6, name=f"std_{tag}")
        nc.scalar.activation(std[:], var[:], AF.Sqrt, bias=c["eps_sc"][:])
        rstd = pool.tile([1, NF], FP32, tag="lrow", bufs=6,
                         name=f"rstd_{tag}")
        nc.vector.reciprocal_approx_fast(rstd[:], std[:])
        mur = pool.tile([1, NF], FP32, tag="lrow", bufs=6, name=f"mur_{tag}")
        nc.vector.tensor_copy(_r(mur[:]), mu[:])
        rstdr = pool.tile([1, NF], FP32, tag="lrow", bufs=6,
                          name=f"rstdr_{tag}")
        nc.vector.tensor_copy(_r(rstdr[:]), rstd[:])
        mub = pool.tile([PC, NF], FP32, tag="lnb", bufs=2, name=f"mub_{tag}")
        bb = psp.tile([PC, NF], FP32, tag="lnbc", bufs=2, name=f"mubp_{tag}")
        nc.tensor.matmul(bb[:], _r(ones1[:]), _r(mur[:]), start=True,
                         stop=True)
        nc.vector.tensor_copy(mub[:], bb[:])
        rstdb = pool.tile([PC, NF], FP32, tag="lnb", bufs=2,
                          name=f"rsb_{tag}")
        bb2 = psp.tile([PC, NF], FP32, tag="lnbc", bufs=2, name=f"rsbp_{tag}")
        nc.tensor.matmul(bb2[:], _r(ones1[:]), _r(rstdr[:]), start=True,
                         stop=True)
        nc.vector.tensor_copy(rstdb[:], bb2[:])
        for j in range(8):
            xj = x_all[:, j * NF:(j + 1) * NF]
            t1 = pool.tile([PC, NF], FP32, tag="lntmp", bufs=2,
                           name=f"lt_{tag}{j}")
            nc.vector.tensor_sub(t1[:], xj, mub[:])
            nc.vector.tensor_mul(t1[:], t1[:], rstdb[:])
            nc.scalar.activation(
                _r(out_tiles[j][:]), t1[:], AF.Identity,
                bias=betaT[:, j:j + 1], scale=gammaT[:, j:j + 1])


def _emit(nc, tc, dram, y_out):
    with tc.tile_pool(name="persist", bufs=1) as pp:
        ones_sc = pp.tile([PC, 8], FP32, tag="ones_sc")
        nc.gpsimd.memset(ones_sc[:], 1.0)
        ones_row_raw = pp.tile([1, PC], FP32, tag="ones_row_raw")
        nc.gpsimd.memset(ones_row_raw[:], 1.0)
        eps_sc = pp.tile([1, 1], FP32, tag="eps_sc")
        nc.gpsimd.memset(eps_sc[:], EPS)
        ones128 = pp.tile([PC, 1], FP32, tag="ones128")
        nc.scalar.copy(_r(ones128[:]), ones_sc[:, 0:1])
        ones1 = pp.tile([1, PC], FP32, tag="ones1")
        nc.scalar.copy(_r(ones1[:]), ones_row_raw[:])

        def bias_cols(name, n):
            t = pp.tile([PC, n], FP32, tag=f"bc_{name}")
            nc.sync.dma_start(t[:], dram[name].ap())
            return t

        bqT = bias_cols("bqc", 4)
        bkT = bias_cols("bkc", 4)
        bq2T = bias_cols("bq2c", 4)
        bk2T = bias_cols("bk2c", 4)
        gammaT = bias_cols("gammac", 8)
        betaT = bias_cols("betac", 8)
        b1T = bias_cols("b1c", FC)
        b2T = bias_cols("b2c", 8)

        c = dict(ones_sc=ones_sc, ones128=ones128, ones1=ones1,
                 gammaT=gammaT, betaT=betaT, eps_sc=eps_sc)

        with tc.tile_pool(name="xdram", bufs=1, space="DRAM") as dp:
            ag_in = dp.tile([N, NF], FP32, name="agin")
            ag_out = dp.tile([2 * N, NF], FP32, name="agout")

            # carry pool: tiles that live across stage boundaries
            with tc.tile_pool(name="carry", bufs=1) as cp:
                nTo = [cp.tile([PC, NF], FP32, tag="nTo", bufs=8,
                               name=f"nTo{i}") for i in range(8)]
                n3T = [cp.tile([PC, NF], FP32, tag="n3T", bufs=8,
                               name=f"n3T{i}") for i in range(8)]

                # ---------------- stage 1 ----------------
                with tc.tile_pool(name="s1b", bufs=1) as s1b:
                    qT = [s1b.tile([PC, N], FP32, tag="qT", bufs=4,
                                   name=f"qT{i}") for i in range(4)]
                    kT = [s1b.tile([PC, N], FP32, tag="kT", bufs=4,
                                   name=f"kT{i}") for i in range(4)]
                    v_tiles = [s1b.tile([PC, 128 * HPC], BF16, tag="v",
                                        bufs=8, name=f"v{i}")
                               for i in range(8)]
                    xT_all = s1b.tile([PC, 8 * NF], FP32, tag="xTa", bufs=1,
                                      name="xT_all")
                    x2own = [s1b.tile([PC, NF], FP32, tag="x2o", bufs=8,
                                      name=f"x2o{i}") for i in range(8)]
                    for j in range(8):
                        nc.sync.dma_start(
                            _r(x2own[j][:]),
                            _r(dram["x2o"].ap()[j * PC:(j + 1) * PC, :]))

                    with tc.tile_pool(name="s1x", bufs=1) as sx:
                        def stream_x(dname):
                            def fn(kc):
                                t = sx.tile([PC, N], FP32, tag="xs",
                                            bufs=3, name=f"xs_{dname}{kc}")
                                nc.sync.dma_start(
                                    _r(t[:]),
                                    _r(dram[dname].ap()[
                                        kc * PC:(kc + 1) * PC, :]))
                                return t[:]
                            return fn

                        _proj_qk(nc, tc, sx, dram["wqt"], bqT,
                                 stream_x("x2t"), qT, "q")
                        _proj_qk(nc, tc, sx, dram["wkt"], bkT,
                                 stream_x("x1t"), kT, "k")
                        _proj_v(nc, tc, sx, dram["wv"], dram["bv"],
                                stream_x("x1t"), v_tiles, "v1",
                                ones_sc, ones1)

                    if KDEBUG:
                        for m in range(4):
                            nc.sync.dma_start(
                                dram["dbg_qT"].ap()[m * PC:(m + 1) * PC, :],
                                qT[m][:])
                            nc.sync.dma_start(
                                dram["dbg_kT"].ap()[m * PC:(m + 1) * PC, :],
                                kT[m][:])
                        for pc in range(8):
                            vf = s1b.tile([PC, NF], FP32, tag="vdbg",
                                          bufs=2, name=f"vdbg{pc}")
                            nc.vector.tensor_copy(
                                vf[:].rearrange("p (h c) -> p h c", h=HPC),
                                v_tiles[pc][:].rearrange(
                                    "p (h c) -> p h c", h=HPC)[:, :, 64:128])
                            nc.sync.dma_start(
                                dram["dbg_v"].ap()[pc * PC:(pc + 1) * PC, :],
                                vf[:])

                    _attention(nc, tc, s1b, qT, kT, v_tiles, xT_all[:],
                               ones1, "x")
                    for j in range(8):
                        nc.vector.tensor_tensor(
                            _r(xT_all[:, j * NF:(j + 1) * NF]),
                            xT_all[:, j * NF:(j + 1) * NF],
                            x2own[j][:], op=ALU.add)
                    if KDEBUG:
                        nc.sync.dma_start(dram["dbg_x1a"].ap(), xT_all[:])

                    _layernorm_T(nc, tc, s1b, xT_all[:], nTo, c, "ln1")
                    if KDEBUG:
                        for j in range(8):
                            nc.sync.dma_start(
                                dram["dbg_nTo"].ap()[j * PC:(j + 1) * PC, :],
                                nTo[j][:])
                    for j in range(8):
                        nc.sync.dma_start(ag_in[j * PC:(j + 1) * PC, :],
                                          nTo[j][:])
                    if os.environ.get("KBENCH_NO_CC", "0") == "1":
                        nc.sync.dma_start(ag_out[0:N, :], ag_in[:])
                        nc.sync.dma_start(ag_out[N:2 * N, :], ag_in[:])
                    else:
                        nc.gpsimd.collective_compute(
                            "AllGather", ALU.bypass,
                            replica_groups=[[0, 1], [2, 3], [4, 5], [6, 7]],
                            ins=[ag_in[:]], outs=[ag_out[:]])

                # ---------------- stage 2 ----------------
                with tc.tile_pool(name="s2b", bufs=1) as s2b:
                    # gathered n, natural row order: block 0 = rows 0..511
                    nT = [s2b.tile([PC, N], FP32, tag="nT", bufs=8,
                                   name=f"nT{i}") for i in range(8)]
                    for j in range(8):
                        nc.sync.dma_start(
                            _r(nT[j][:, 0:NF]),
                            _r(ag_out[j * PC:(j + 1) * PC, :]))
                        nc.sync.dma_start(
                            _r(nT[j][:, NF:N]),
                            _r(ag_out[N + j * PC:N + (j + 1) * PC, :]))
                    q2T = [s2b.tile([PC, N], FP32, tag="q2T", bufs=4,
                                    name=f"q2T{i}") for i in range(4)]
                    k2T = [s2b.tile([PC, N], FP32, tag="k2T", bufs=4,
                                    name=f"k2T{i}") for i in range(4)]
                    v2_tiles = [s2b.tile([PC, 128 * HPC], BF16, tag="v2",
                                         bufs=8, name=f"v2_{i}")
                                for i in range(8)]
                    x3_all = s2b.tile([PC, 8 * NF], FP32, tag="x3a", bufs=1,
                                      name="x3_all")
                    _proj_qk(nc, tc, s2b, dram["wq2t"], bq2T,
                             lambda kc: nT[kc][:], q2T, "q2")
                    _proj_qk(nc, tc, s2b, dram["wk2t"], bk2T,
                             lambda kc: nT[kc][:], k2T, "k2")
                    _proj_v(nc, tc, s2b, dram["wv2"], dram["bv2"],
                            lambda kc: nT[kc][:], v2_tiles, "v2",
                            ones_sc, ones1)

                    if KDEBUG:
                        for j in range(8):
                            nc.sync.dma_start(
                                dram["dbg_nT"].ap()[j * PC:(j + 1) * PC, :],
                                nT[j][:])

                    _attention(nc, tc, s2b, q2T, k2T, v2_tiles, x3_all[:],
                               ones1, "y")
                    for j in range(8):
                        nc.vector.tensor_tensor(
                            _r(x3_all[:, j * NF:(j + 1) * NF]),
                            x3_all[:, j * NF:(j + 1) * NF],
                            nTo[j][:], op=ALU.add)
                    if KDEBUG:
                        nc.sync.dma_start(dram["dbg_x3a"].ap(), x3_all[:])

                    _layernorm_T(nc, tc, s2b, x3_all[:], n3T, c, "ln2")
                    if KDEBUG:
                        for j in range(8):
                            nc.sync.dma_start(
                                dram["dbg_n3"].ap()[j * PC:(j + 1) * PC, :],
                                n3T[j][:])

                # ---------------- stage 3 (MLP) ----------------
                with tc.tile_pool(name="s3", bufs=1) as s3:
                    hT = [s3.tile([PC, NF], FP32, tag="hT", bufs=FC,
                                  name=f"hT{i}") for i in range(FC)]
                    with tc.tile_pool(name="s3p", space="PSUM",
                                      bufs=1) as psp:
                        for f in range(FC):
                            wt = s3.tile([PC, D], FP32, tag="w1t",
                                         bufs=4, name=f"w1t{f}")
                            nc.sync.dma_start(
                                _r(wt[:]),
                                _r(dram["w1t"].ap()[f * PC:(f + 1) * PC, :]))
                            ps = psp.tile([PC, NF], FP32, tag="fc1",
                                          bufs=3, name=f"h{f}")
                            for kc in range(KC):
                                nc.tensor.matmul(
                                    ps[:],
                                    _r(wt[:, kc * PC:(kc + 1) * PC]),
                                    _r(n3T[kc][:]), start=(kc == 0),
                                    stop=(kc == KC - 1))
                            nc.scalar.activation(
                                _r(hT[f][:]), ps[:], AF.Gelu,
                                bias=b1T[:, f:f + 1])
                        for d in range(8):
                            w2 = s3.tile([PC, F], FP32, tag="w2t",
                                         bufs=2, name=f"w2t{d}")
                            nc.sync.dma_start(
                                _r(w2[:]),
                                _r(dram["w2t"].ap()[d * PC:(d + 1) * PC, :]))
                            ps = psp.tile([PC, NF], FP32, tag="fc2",
                                          bufs=3, name=f"yp{d}")
                            for f in range(FC):
                                nc.tensor.matmul(
                                    ps[:],
                                    _r(w2[:, f * PC:(f + 1) * PC]),
                                    _r(hT[f][:]), start=(f == 0),
                                    stop=(f == FC - 1))
                            yt = s3.tile([PC, NF], FP32, tag="yT",
                                         bufs=4, name=f"yT{d}")
                            nc.vector.scalar_tensor_tensor(
                                yt[:], ps[:], b2T[:, d:d + 1],
                                n3T[d][:], op0=ALU.add, op1=ALU.add)
                            nc.sync.dma_start(
                                y_out.ap()[d * PC:(d + 1) * PC, :], yt[:])


def _get_nc():
    if "nc" not in _CACHE:
        _CACHE["nc"] = _build()
    return _CACHE["nc"]


def _prep_shared(inputs):
    """Host-side weight rearrangement shared across cores."""
    Wq = _round_fp32r(inputs["Wq"])
    Wkv = _round_fp32r(inputs["Wkv"])
    Wqkv = _round_fp32r(inputs["Wqkv"])
    W1 = _round_fp32r(inputs["W1"])
    W2 = _round_fp32r(inputs["W2"])
    bq = _round_fp32r(inputs["bq"])
    bkv = _round_fp32r(inputs["bkv"])
    bqkv = _round_fp32r(inputs["bqkv"])

    def qk_tiles(Wslice):
        # [1024, 512] -> [4*128, 8*128] with row m*128+p, col kc*128+c
        a = Wslice.reshape(KC, PC, 4, PC)           # (kc, p, m, c)
        return np.ascontiguousarray(
            a.transpose(2, 1, 0, 3).reshape(4 * PC, D))

    sh = {}
    for hh in range(2):
        lo = NF * hh
        sh[hh] = {
            "wqt": qk_tiles(Wq[:, lo:lo + NF]),
            "wkt": qk_tiles(Wkv[:, lo:lo + NF]),
            "wq2t": qk_tiles(Wqkv[:, lo:lo + NF]),
            "wk2t": qk_tiles(Wqkv[:, D + lo:D + lo + NF]),
            "wv": np.ascontiguousarray(Wkv[:, D + lo:D + lo + NF]),
            "wv2": np.ascontiguousarray(Wqkv[:, 2 * D + lo:2 * D + lo + NF]),
            "bqc": np.ascontiguousarray(bq[lo:lo + NF].reshape(4, PC).T),
            "bkc": np.ascontiguousarray(bkv[lo:lo + NF].reshape(4, PC).T),
            "bq2c": np.ascontiguousarray(bqkv[lo:lo + NF].reshape(4, PC).T),
            "bk2c": np.ascontiguousarray(
                bqkv[D + lo:D + lo + NF].reshape(4, PC).T),
            "bv": np.ascontiguousarray(bkv[D + lo:D + lo + NF]),
            "bv2": np.ascontiguousarray(bqkv[2 * D + lo:2 * D + lo + NF]),
        }
    # w1t: row f*128+p, col kc*128+c = W1[kc*128+p, f*128+c]
    w1t = np.ascontiguousarray(
        W1.reshape(KC, PC, FC, PC).transpose(2, 1, 0, 3).reshape(F, D))
    # w2t: row d*128+p, col fc*128+c = W2[fc*128+p, d*128+c]
    w2t = np.ascontiguousarray(
        W2.reshape(FC, PC, 8, PC).transpose(2, 1, 0, 3).reshape(D, F))
    gamma = np.ascontiguousarray(np.asarray(inputs["gamma"], np.float32))
    beta = np.ascontiguousarray(np.asarray(inputs["beta"], np.float32))
    b1 = np.ascontiguousarray(np.asarray(inputs["b1"], np.float32))
    b2 = np.ascontiguousarray(np.asarray(inputs["b2"], np.float32))
    common = {
        "w1t": w1t, "w2t": w2t,
        "gammac": np.ascontiguousarray(gamma.reshape(8, PC).T),
        "betac": np.ascontiguousarray(beta.reshape(8, PC).T),
        "b1c": np.ascontiguousarray(b1.reshape(FC, PC).T),
        "b2c": np.ascontiguousarray(b2.reshape(8, PC).T),
    }
    return sh, common


def kernel(**inputs):
    x1 = np.asarray(inputs["x1"], np.float32)
    x2 = np.asarray(inputs["x2"], np.float32)
    sh, common = _prep_shared(inputs)
    x1t = [_round_fp32r(np.ascontiguousarray(x1[b].T)) for b in range(B)]
    x2t = [_round_fp32r(np.ascontiguousarray(x2[b].T)) for b in range(B)]

    nc = _get_nc()
    in_maps = []
    for core in range(8):
        b, hh = core // 2, core % 2
        lo = NF * hh
        m = {"x1t": x1t[b], "x2t": x2t[b],
             "x2o": np.ascontiguousarray(x2t[b][:, lo:lo + NF])}
        m.update(sh[hh])
        m.update(common)
        in_maps.append(m)
    res = run_bass_kernel_spmd(nc, in_maps, core_ids=list(range(8)))
    _CACHE["last_results"] = res
    out = np.zeros((B, N, D), np.float32)
    for core in range(8):
        b, hh = core // 2, core % 2
        out[b, NF * hh:NF * hh + NF, :] = res.results[core]["y"].T
    return out


# revision 30
# speedup vs baseline: 1.4818x; 1.1827x over previous
"""Trainium2 Bass kernel for nn_CrossSelfDecoder (B=4,N=1024,D=1024,H=16,F=4096).

Sharding: 8 cores = (batch b in 0..3) x (head-half hh in 0..1). Each core
computes attention for its 8 heads over all 1024 positions of its batch.
Because the reference reshapes (B,H,N,Dp)->(B,N,D) without permuting heads
back, head-ownership makes row-ownership invariant: core (b,hh) owns rows
[512*hh, 512*hh+512) of batch b through the whole network.

v2 layout strategy vs v1:
- x1/x2 are pre-transposed on the HOST (numpy) and uploaded as [D, N], so
  no PE transposes / PSUM->SBUF copies are needed on-chip. The output is
  produced transposed [D, NF] and transposed back on the host.
- Weights are pre-arranged on the host into the exact SBUF tile layouts so
  every weight DMA is a contiguous [128, X] block read.
- Softmax exp runs on 2-bank PSUM tiles [128, 1024] (both query halves per
  (head, key-block)), halving ACT instruction count.
- Softmax P and V tiles are bf16 (pure-bf16 PV matmuls, full PE rate).
- Attention normalize uses reciprocal_approx_fast + two merged [64, 8x32]
  strided DVE multiplies per (head, qh) instead of 16 tiny ops.
- Projection bias-adds moved to DVE (tensor_scalar_add).
- LN1 output round-trips through the AllGather in natural row order (rank
  order [even, odd] == row order), so one compiled program serves all
  cores; LN2 output stays in SBUF (no DRAM round trip).
"""

import os
import numpy as np

import concourse.mybir as mybir
import concourse.tile as tile
from concourse import bacc
from concourse.bass_utils import run_bass_kernel_spmd

FP32 = mybir.dt.float32
FP32R = mybir.dt.float32r
BF16 = mybir.dt.bfloat16
AF = mybir.ActivationFunctionType
ALU = mybir.AluOpType

B, N, D, H, F = 4, 1024, 1024, 16, 4096
Dp = D // H           # 64
HPC = 8               # heads per core
PC = 128              # partition chunk
NF = 512              # free chunk (one psum bank of fp32)
KC = D // PC          # 8 contraction chunks
FC = F // PC          # 32
EPS = 1e-5

_CACHE = {}


def _round_fp32r(x):
    """Round-to-nearest-even onto fp32r's 1+8+11-bit grid (top 20 bits)."""
    x = np.ascontiguousarray(x, dtype=np.float32)
    b = x.view(np.uint32)
    low = b & np.uint32(0xFFF)
    half = np.uint32(0x800)
    bump = (low > half) | (
        (low == half) & (((b >> np.uint32(12)) & np.uint32(1)) != 0)
    )
    out = (b & np.uint32(0xFFFFF000)) + np.where(
        bump, np.uint32(0x1000), np.uint32(0)
    ).astype(np.uint32)
    return out.view(np.float32).copy()


def _r(ap):
    return ap.bitcast(FP32R)


KDEBUG = os.environ.get("KDEBUG", "0") == "1"
_dbg_dram = {}


def _build():
    global _dbg_dram
    nc = bacc.Bacc("TRN2", target_bir_lowering=False, debug=False,
                   num_devices=8)
    dram = {}
    _dbg_dram = dram
    if KDEBUG:
        for nm, shp in [
            ("dbg_qT", [4 * PC, N]), ("dbg_kT", [4 * PC, N]),
            ("dbg_x1a", [PC, 8 * NF]), ("dbg_nTo", [N, NF]),
            ("dbg_nT", [N, N]), ("dbg_x3a", [PC, 8 * NF]),
            ("dbg_n3", [N, NF]), ("dbg_v", [N, NF]),
            ("dbg_pt", [PC, N]), ("dbg_ops", [PC, NF]),
        ]:
            dram[nm] = nc.dram_tensor(nm, shp, FP32, kind="ExternalOutput")
    bf16_inputs = {"x1t", "x2t", "wqt", "wkt", "wq2t", "wk2t",
                   "wv", "wv2", "w1t", "w2t"}
    for nm, shp in [
        ("x1t", [D, N]), ("x2t", [D, N]), ("x2o", [D, NF]),
        # wq_t/wk_t: [4*128, 8*128]; row m*128+p, col kc*128+c =
        #   W[kc*128+p, m*128+c] (own 512-col slice)
        ("wqt", [4 * PC, D]), ("wkt", [4 * PC, D]),
        ("wq2t", [4 * PC, D]), ("wk2t", [4 * PC, D]),
        # wv: natural [D, 512] own-col slice
        ("wv", [D, NF]), ("wv2", [D, NF]),
        # w1_t: [32*128, 8*128]; row f*128+p, col kc*128+c = W1[kc*128+p, f*128+c]
        ("w1t", [F, D]),
        # w2_t: [8*128, 32*128]; row d*128+p, col fc*128+c = W2[fc*128+p, d*128+c]
        ("w2t", [D, F]),
        ("bqc", [PC, 4]), ("bkc", [PC, 4]),
        ("bq2c", [PC, 4]), ("bk2c", [PC, 4]),
        ("bv", [NF]), ("bv2", [NF]),
        ("gammac", [PC, 8]), ("betac", [PC, 8]),
        ("b1c", [PC, FC]), ("b2c", [PC, 8]),
    ]:
        dram[nm] = nc.dram_tensor(
            nm, shp, BF16 if nm in bf16_inputs else FP32,
            kind="ExternalInput")
    y_out = nc.dram_tensor("y", [D, NF], FP32, kind="ExternalOutput")

    with tile.TileContext(nc) as tc:
        _emit(nc, tc, dram, y_out)
    nc.compile()
    return nc


def _proj_qk(nc, tc, pool, w_dram, bias_tile, rhs_fn, out_tiles, tag):
    """T-domain projection: out[m] [128, 1024] fp32r = W.T @ x + bias.

    kc-outer loop so the rhs activation chunks can be streamed.
    rhs_fn(kc) -> [128, 1024] AP (feature chunk kc, all 1024 rows).
    """
    with tc.tile_pool(name=f"proj_{tag}", space="PSUM", bufs=1) as psp:
        wts, pss = [], []
        for m in range(4):
            wt = pool.tile([PC, D], BF16, tag="wqk", bufs=4,
                           name=f"w_{tag}{m}")
            nc.sync.dma_start(wt[:], w_dram.ap()[m * PC:(m + 1) * PC, :])
            wts.append(wt)
            pss.append(psp.tile([PC, N], FP32, tag="proj", bufs=4,
                                name=f"p_{tag}{m}"))
        for kc in range(KC):
            rhs = rhs_fn(kc)
            for m in range(4):
                w = wts[m][:, kc * PC:(kc + 1) * PC]
                nc.tensor.matmul(pss[m][:, 0:NF], w, rhs[:, 0:NF],
                                 start=(kc == 0), stop=(kc == KC - 1))
                nc.tensor.matmul(pss[m][:, NF:N], w, rhs[:, NF:N],
                                 start=(kc == 0), stop=(kc == KC - 1))
        for m in range(4):
            nc.vector.tensor_scalar_add(out_tiles[m][:], pss[m][:],
                                        bias_tile[:, m:m + 1])


def _proj_v(nc, tc, pool, w_dram, bias_row_dram, lhsT_fn, v_tiles, tag,
            ones_sc, ones1):
    """v natural (1024 rows x 512 own-head cols) + per-head ones column.
    v_tiles: 8 x (128, 520) bf16: head h cols [65h,65h+64), col 65h+64=1.
    lhsT_fn(kc) -> [128, 1024] AP (feature chunk kc on partitions, rows on
    free); slices of it are the per-pc stationary operands. Bias is folded
    into PSUM as a K=1 broadcast matmul."""
    bvR = pool.tile([1, NF], FP32, tag="bvr", bufs=2, name=f"bvR_{tag}")
    nc.sync.dma_start(_r(bvR[:]), _r(bias_row_dram.ap().unsqueeze(0)))
    with tc.tile_pool(name=f"projv_{tag}", space="PSUM", bufs=1) as psp:
        pss = [psp.tile([PC, NF], FP32, tag="vproj", bufs=8,
                        name=f"v_{tag}{pc}") for pc in range(8)]
        for pc in range(8):
            nc.tensor.matmul(pss[pc][:], _r(ones1[:]), _r(bvR[:]),
                             start=True, stop=False)
        for kc in range(KC):
            wt = pool.tile([PC, NF], BF16, tag="wvs", bufs=3,
                           name=f"wv_{tag}{kc}")
            nc.sync.dma_start(wt[:], w_dram.ap()[kc * PC:(kc + 1) * PC, :])
            lhsT = lhsT_fn(kc)
            for pc in range(8):
                nc.tensor.matmul(pss[pc][:],
                                 lhsT[:, pc * PC:(pc + 1) * PC],
                                 wt[:], start=False, stop=(kc == KC - 1))
        for pc in range(8):
            # head block: col 96h = ones (denominator -> PSUM partition 0),
            # cols 96h+32..96h+96 = data (PSUM partitions 32..96, aligned)
            vt3 = v_tiles[pc][:].rearrange("p (h c) -> p h c", h=HPC)
            nc.vector.tensor_copy(
                vt3[:, :, 64:128],
                pss[pc][:].rearrange("p (h c) -> p h c", h=HPC))
            nc.scalar.copy(vt3[:, :, 0:1].squeeze(2), ones_sc[:])


def _attention(nc, tc, pool, qT, kT, v_tiles, xT_all, ones1, ones128,
               warm_rhs, tag):
    """Own-head attention, normalized + scrambled into xT_all [128, 4096].

    xT_all[64*mm + d, 512*j + 64*hloc + 32*qh + u] =
        O_norm[hloc][d, q = 512*qh + 16*u + 2*j + mm]
    """
    with tc.tile_pool(name=f"attn_{tag}", space="PSUM", bufs=1) as psp:
        warm = psp.tile([1, NF], FP32, tag="warm", bufs=1,
                        name=f"warm_{tag}")

        def keep_warm(n):
            # HAM keeps the PE at 1.2 GHz unless it sees sustained busy
            # windows; the exp-gated attention stream has micro-gaps that
            # never qualify. These throwaway matmuls fill the gaps.
            for _ in range(n):
                nc.tensor.matmul(warm[:], _r(ones128[:]),
                                 _r(warm_rhs[:, 0:NF]), start=True,
                                 stop=True)

        for hloc in range(HPC):
            t4, r64 = hloc // 2, Dp * (hloc % 2)
            opss = [psp.tile([PC, NF], FP32, tag="O", bufs=2,
                             name=f"O_{tag}{hloc}_{qh}") for qh in range(2)]
            for kc in range(KC):
                sps = psp.tile([PC, N], FP32, tag="S", bufs=2,
                               name=f"S_{tag}{hloc}_{kc}")
                for qh in range(2):
                    nc.tensor.matmul(
                        sps[:, qh * NF:(qh + 1) * NF],
                        kT[t4][r64:r64 + Dp, kc * PC:(kc + 1) * PC],
                        qT[t4][r64:r64 + Dp, qh * NF:(qh + 1) * NF],
                        start=True, stop=True)
                pt = pool.tile([PC, N], BF16, tag="PT", bufs=3,
                               name=f"PT_{tag}{hloc}_{kc}")
                nc.scalar.activation(pt[:], sps[:], AF.Exp)
                if KDEBUG and tag == "x" and hloc == 0 and kc == 0:
                    ptf = pool.tile([PC, N], FP32, tag="ptdbg", bufs=1,
                                    name="ptdbg")
                    nc.vector.tensor_copy(ptf[:], pt[:])
                    nc.sync.dma_start(
                        _dbg_dram["dbg_pt"].ap(), ptf[:])
                for qh in range(2):
                    nc.tensor.matmul(
                        opss[qh][:],
                        v_tiles[kc][:, 128 * hloc:128 * hloc + 128],
                        pt[:, qh * NF:(qh + 1) * NF],
                        start=(kc == 0), stop=(kc == KC - 1))
                keep_warm(1)
            if KDEBUG and tag == "x" and hloc == 0:
                of = pool.tile([PC, NF], FP32, tag="opsdbg", bufs=1,
                               name="opsdbg")
                nc.vector.tensor_copy(of[:], opss[0][:])
                nc.sync.dma_start(_dbg_dram["dbg_ops"].ap(), of[:])
            for qh in range(2):
                ops = opss[qh]
                # row 0 = denominator (ones col first); rows 1..64 = data.
                # reciprocal_approx_fast (custom DVE) requires its input at
                # partition offset 0 — the offset is dropped otherwise.
                rrow = pool.tile([1, NF], FP32, tag="rrow", bufs=2,
                                 name=f"rr_{tag}{hloc}_{qh}")
                nc.vector.reciprocal_approx_fast(rrow[:], ops[0:1, :])
                rrowr = pool.tile([1, NF], FP32, tag="rrowr", bufs=2,
                                  name=f"rrr_{tag}{hloc}_{qh}")
                nc.vector.tensor_copy(_r(rrowr[:]), rrow[:])
                rbp = psp.tile([Dp, NF], FP32, tag="rb", bufs=1,
                               name=f"rbp_{tag}{hloc}_{qh}")
                nc.tensor.matmul(rbp[:], _r(ones1[:, 0:Dp]), _r(rrowr[:]),
                                 start=True, stop=True)
                rb = pool.tile([Dp, NF], FP32, tag="rbs", bufs=2,
                               name=f"rb_{tag}{hloc}_{qh}")
                nc.vector.tensor_copy(rb[:], rbp[:])
                keep_warm(2)
                # merged scramble-normalize: 2 ops of [64, (j:8, u:32)]
                src4 = ops[64:128, :].rearrange(
                    "d (u j2 m2) -> d j2 u m2", u=32, j2=8)
                rb4 = rb[:].rearrange("d (u j2 m2) -> d j2 u m2", u=32, j2=8)
                for mm in range(2):
                    dst = xT_all[Dp * mm:Dp * mm + Dp, :].rearrange(
                        "d (j r) -> d j r", j=8)[
                        :, :, Dp * hloc + 32 * qh:Dp * hloc + 32 * qh + 32]
                    nc.vector.tensor_tensor(
                        _r(dst), src4[:, :, :, mm], rb4[:, :, :, mm],
                        op=ALU.mult)


def _layernorm_T(nc, tc, pool, x_all, out_tiles, c, tag):
    """out[j] [128, 512] fp32r = LN(x_all [128, 4096]) over features
    (partitions x 8 chunks)."""
    ones128, ones1 = c["ones128"], c["ones1"]
    gammaT, betaT = c["gammaT"], c["betaT"]
    with tc.tile_pool(name=f"ln_{tag}", space="PSUM", bufs=1) as psp:
        s0 = psp.tile([1, NF], FP32, tag="s0", bufs=1, name=f"s0_{tag}")
        s1 = psp.tile([1, NF], FP32, tag="s1", bufs=1, name=f"s1_{tag}")
        for j in range(8):
            xj = x_all[:, j * NF:(j + 1) * NF]
            nc.tensor.matmul(s0[:], _r(ones128[:]), _r(xj), start=(j == 0),
                             stop=(j == 7))
            sq = pool.tile([PC, NF], FP32, tag="sq", bufs=2,
                           name=f"sq_{tag}{j}")
            nc.vector.tensor_tensor(_r(sq[:]), xj, xj, op=ALU.mult)
            nc.tensor.matmul(s1[:], _r(ones128[:]), _r(sq[:]),
                             start=(j == 0), stop=(j == 7))
        mu = pool.tile([1, NF], FP32, tag="lrow", bufs=6, name=f"mu_{tag}")
        nc.scalar.mul(mu[:], s0[:], 1.0 / D)
        msq = pool.tile([1, NF], FP32, tag="lrow", bufs=6, name=f"msq_{tag}")
        nc.scalar.mul(msq[:], s1[:], 1.0 / D)
        mu2 = pool.tile([1, NF], FP32, tag="lrow", bufs=6, name=f"mu2_{tag}")
        nc.scalar.square(mu2[:], mu[:])
        var = pool.tile([1, NF], FP32, tag="lrow", bufs=6, name=f"var_{tag}")
        nc.vector.tensor_sub(var[:], msq[:], mu2[:])
        std = pool.tile([1, NF], FP32, tag="lrow", bufs=6, name=f"std_{tag}")
        nc.scalar.activation(std[:], var[:], AF.Sqrt, bias=c["eps_sc"][:])
        rstd = pool.tile([1, NF], FP32, tag="lrow", bufs=6,
                         name=f"rstd_{tag}")
        nc.vector.reciprocal_approx_fast(rstd[:], std[:])
        mur = pool.tile([1, NF], FP32, tag="lrow", bufs=6, name=f"mur_{tag}")
        nc.vector.tensor_copy(_r(mur[:]), mu[:])
        rstdr = pool.tile([1, NF], FP32, tag="lrow", bufs=6,
                          name=f"rstdr_{tag}")
        nc.vector.tensor_copy(_r(rstdr[:]), rstd[:])
        mub = pool.tile([PC, NF], FP32, tag="lnb", bufs=2, name=f"mub_{tag}")
        bb = psp.tile([PC, NF], FP32, tag="lnbc", bufs=2, name=f"mubp_{tag}")
        nc.tensor.matmul(bb[:], _r(ones1[:]), _r(mur[:]), start=True,
                         stop=True)
        nc.vector.tensor_copy(mub[:], bb[:])
        rstdb = pool.tile([PC, NF], FP32, tag="lnb", bufs=2,
                          name=f"rsb_{tag}")
        bb2 = psp.tile([PC, NF], FP32, tag="lnbc", bufs=2, name=f"rsbp_{tag}")
        nc.tensor.matmul(bb2[:], _r(ones1[:]), _r(rstdr[:]), start=True,
                         stop=True)
        nc.vector.tensor_copy(rstdb[:], bb2[:])
        for j in range(8):
            xj = x_all[:, j * NF:(j + 1) * NF]
            t1 = pool.tile([PC, NF], FP32, tag="lntmp", bufs=2,
                           name=f"lt_{tag}{j}")
            nc.vector.tensor_sub(t1[:], xj, mub[:])
            nc.vector.tensor_mul(t1[:], t1[:], rstdb[:])
            nc.scalar.activation(
                _r(out_tiles[j][:]), t1[:], AF.Identity,
                bias=betaT[:, j:j + 1], scale=gammaT[:, j:j + 1])


def _emit(nc, tc, dram, y_out):
    with tc.tile_pool(name="persist", bufs=1) as pp:
        ones_sc = pp.tile([PC, 8], FP32, tag="ones_sc")
        nc.gpsimd.memset(ones_sc[:], 1.0)
        ones_row_raw = pp.tile([1, PC], FP32, tag="ones_row_raw")
        nc.gpsimd.memset(ones_row_raw[:], 1.0)
        eps_sc = pp.tile([1, 1], FP32, tag="eps_sc")
        nc.gpsimd.memset(eps_sc[:], EPS)
        ones128 = pp.tile([PC, 1], FP32, tag="ones128")
        nc.scalar.copy(_r(ones128[:]), ones_sc[:, 0:1])
        ones1 = pp.tile([1, PC], FP32, tag="ones1")
        nc.scalar.copy(_r(ones1[:]), ones_row_raw[:])

        def bias_cols(name, n):
            t = pp.tile([PC, n], FP32, tag=f"bc_{name}")
            nc.sync.dma_start(t[:], dram[name].ap())
            return t

        bqT = bias_cols("bqc", 4)
        bkT = bias_cols("bkc", 4)
        bq2T = bias_cols("bq2c", 4)
        bk2T = bias_cols("bk2c", 4)
        gammaT = bias_cols("gammac", 8)
        betaT = bias_cols("betac", 8)
        b1T = bias_cols("b1c", FC)
        b2T = bias_cols("b2c", 8)

        c = dict(ones_sc=ones_sc, ones128=ones128, ones1=ones1,
                 gammaT=gammaT, betaT=betaT, eps_sc=eps_sc)

        with tc.tile_pool(name="xdram", bufs=1, space="DRAM") as dp:
            ag_in = dp.tile([N, NF], BF16, name="agin")
            ag_out = dp.tile([2 * N, NF], BF16, name="agout")

            # carry pool: tiles that live across stage boundaries
            with tc.tile_pool(name="carry", bufs=1) as cp:
                nTo = [cp.tile([PC, NF], FP32, tag="nTo", bufs=8,
                               name=f"nTo{i}") for i in range(8)]
                n3T = [cp.tile([PC, NF], FP32, tag="n3T", bufs=8,
                               name=f"n3T{i}") for i in range(8)]

                # ---------------- stage 1 ----------------
                with tc.tile_pool(name="s1b", bufs=1) as s1b:
                    qT = [s1b.tile([PC, N], BF16, tag="qT", bufs=4,
                                   name=f"qT{i}") for i in range(4)]
                    kT = [s1b.tile([PC, N], BF16, tag="kT", bufs=4,
                                   name=f"kT{i}") for i in range(4)]
                    v_tiles = [s1b.tile([PC, 128 * HPC], BF16, tag="v",
                                        bufs=8, name=f"v{i}")
                               for i in range(8)]
                    xT_all = s1b.tile([PC, 8 * NF], FP32, tag="xTa", bufs=1,
                                      name="xT_all")
                    x2own = [s1b.tile([PC, NF], FP32, tag="x2o", bufs=8,
                                      name=f"x2o{i}") for i in range(8)]
                    for j in range(8):
                        nc.sync.dma_start(
                            _r(x2own[j][:]),
                            _r(dram["x2o"].ap()[j * PC:(j + 1) * PC, :]))

                    with tc.tile_pool(name="s1x", bufs=1) as sx:
                        def stream_x(dname):
                            def fn(kc):
                                t = sx.tile([PC, N], BF16, tag="xs",
                                            bufs=3, name=f"xs_{dname}{kc}")
                                nc.sync.dma_start(
                                    t[:],
                                    dram[dname].ap()[
                                        kc * PC:(kc + 1) * PC, :])
                                return t[:]
                            return fn

                        _proj_qk(nc, tc, sx, dram["wqt"], bqT,
                                 stream_x("x2t"), qT, "q")
                        _proj_qk(nc, tc, sx, dram["wkt"], bkT,
                                 stream_x("x1t"), kT, "k")
                        _proj_v(nc, tc, sx, dram["wv"], dram["bv"],
                                stream_x("x1t"), v_tiles, "v1",
                                ones_sc, ones1)

                    if KDEBUG:
                        for m in range(4):
                            nc.sync.dma_start(
                                dram["dbg_qT"].ap()[m * PC:(m + 1) * PC, :],
                                qT[m][:])
                            nc.sync.dma_start(
                                dram["dbg_kT"].ap()[m * PC:(m + 1) * PC, :],
                                kT[m][:])
                        for pc in range(8):
                            vf = s1b.tile([PC, NF], FP32, tag="vdbg",
                                          bufs=2, name=f"vdbg{pc}")
                            nc.vector.tensor_copy(
                                vf[:].rearrange("p (h c) -> p h c", h=HPC),
                                v_tiles[pc][:].rearrange(
                                    "p (h c) -> p h c", h=HPC)[:, :, 64:128])
                            nc.sync.dma_start(
                                dram["dbg_v"].ap()[pc * PC:(pc + 1) * PC, :],
                                vf[:])

                    _attention(nc, tc, s1b, qT, kT, v_tiles, xT_all[:],
                               ones1, ones128, x2own[0][:], "x")
                    for j in range(8):
                        nc.vector.tensor_tensor(
                            _r(xT_all[:, j * NF:(j + 1) * NF]),
                            xT_all[:, j * NF:(j + 1) * NF],
                            x2own[j][:], op=ALU.add)
                    if KDEBUG:
                        nc.sync.dma_start(dram["dbg_x1a"].ap(), xT_all[:])

                    _layernorm_T(nc, tc, s1b, xT_all[:], nTo, c, "ln1")
                    if KDEBUG:
                        for j in range(8):
                            nc.sync.dma_start(
                                dram["dbg_nTo"].ap()[j * PC:(j + 1) * PC, :],
                                nTo[j][:])
                    for j in range(8):
                        nTob = s1b.tile([PC, NF], BF16, tag="nTob", bufs=2,
                                        name=f"nTob{j}")
                        nc.vector.tensor_copy(nTob[:], nTo[j][:])
                        nc.sync.dma_start(ag_in[j * PC:(j + 1) * PC, :],
                                          nTob[:])
                    if os.environ.get("KBENCH_NO_CC", "0") == "1":
                        nc.sync.dma_start(ag_out[0:N, :], ag_in[:])
                        nc.sync.dma_start(ag_out[N:2 * N, :], ag_in[:])
                    else:
                        nc.gpsimd.collective_compute(
                            "AllGather", ALU.bypass,
                            replica_groups=[[0, 1], [2, 3], [4, 5], [6, 7]],
                            ins=[ag_in[:]], outs=[ag_out[:]])

                # ---- MLP weight prefetch (fills the collective gap) ----
                w1ts = []
                with tc.tile_pool(name="mlpw", bufs=1) as mw:
                    for f in range(FC // 2):
                        wt = mw.tile([PC, D], BF16, tag="w1t", bufs=FC // 2,
                                     name=f"w1t{f}")
                        nc.sync.dma_start(
                            wt[:], dram["w1t"].ap()[f * PC:(f + 1) * PC, :])
                        w1ts.append(wt)
                    _stage23(nc, tc, dram, c, ag_out, nTo, n3T, w1ts,
                             bq2T, bk2T, b1T, b2T, ones_sc, ones1, ones128,
                             y_out)


def _stage23(nc, tc, dram, c, ag_out, nTo, n3T, w1ts, bq2T, bk2T, b1T,
             b2T, ones_sc, ones1, ones128, y_out):
    if True:
        if True:
            if True:
                # ---------------- stage 2 ----------------
                with tc.tile_pool(name="s2b", bufs=1) as s2b:
                    # gathered n, natural row order: block 0 = rows 0..511
                    nT = [s2b.tile([PC, N], BF16, tag="nT", bufs=8,
                                   name=f"nT{i}") for i in range(8)]
                    for j in range(8):
                        nc.sync.dma_start(
                            nT[j][:, 0:NF],
                            ag_out[j * PC:(j + 1) * PC, :])
                        nc.sync.dma_start(
                            nT[j][:, NF:N],
                            ag_out[N + j * PC:N + (j + 1) * PC, :])
                    q2T = [s2b.tile([PC, N], BF16, tag="q2T", bufs=4,
                                    name=f"q2T{i}") for i in range(4)]
                    k2T = [s2b.tile([PC, N], BF16, tag="k2T", bufs=4,
                                    name=f"k2T{i}") for i in range(4)]
                    v2_tiles = [s2b.tile([PC, 128 * HPC], BF16, tag="v2",
                                         bufs=8, name=f"v2_{i}")
                                for i in range(8)]
                    x3_all = s2b.tile([PC, 8 * NF], FP32, tag="x3a", bufs=1,
                                      name="x3_all")
                    _proj_qk(nc, tc, s2b, dram["wq2t"], bq2T,
                             lambda kc: nT[kc][:], q2T, "q2")
                    _proj_qk(nc, tc, s2b, dram["wk2t"], bk2T,
                             lambda kc: nT[kc][:], k2T, "k2")
                    _proj_v(nc, tc, s2b, dram["wv2"], dram["bv2"],
                            lambda kc: nT[kc][:], v2_tiles, "v2",
                            ones_sc, ones1)

                    if KDEBUG:
                        for j in range(8):
                            nc.sync.dma_start(
                                dram["dbg_nT"].ap()[j * PC:(j + 1) * PC, :],
                                nT[j][:])

                    _attention(nc, tc, s2b, q2T, k2T, v2_tiles, x3_all[:],
                               ones1, ones128, nTo[0][:], "y")
                    for j in range(8):
                        nc.vector.tensor_tensor(
                            _r(x3_all[:, j * NF:(j + 1) * NF]),
                            x3_all[:, j * NF:(j + 1) * NF],
                            nTo[j][:], op=ALU.add)
                    if KDEBUG:
                        nc.sync.dma_start(dram["dbg_x3a"].ap(), x3_all[:])

                    _layernorm_T(nc, tc, s2b, x3_all[:], n3T, c, "ln2")
                    if KDEBUG:
                        for j in range(8):
                            nc.sync.dma_start(
                                dram["dbg_n3"].ap()[j * PC:(j + 1) * PC, :],
                                n3T[j][:])

                # ---------------- stage 3 (MLP) ----------------
                with tc.tile_pool(name="s3", bufs=1) as s3:
                    n3b = [s3.tile([PC, NF], BF16, tag="n3b", bufs=8,
                                   name=f"n3b{i}") for i in range(8)]
                    for j in range(8):
                        nc.vector.tensor_copy(n3b[j][:], n3T[j][:])
                    hT = [s3.tile([PC, NF], BF16, tag="hT", bufs=FC,
                                  name=f"hT{i}") for i in range(FC)]
                    with tc.tile_pool(name="s3p", space="PSUM",
                                      bufs=1) as psp:
                        for f in range(FC):
                            if f < FC // 2:
                                wt = w1ts[f]
                            else:
                                wt = s3.tile([PC, D], BF16, tag="w1s",
                                             bufs=4, name=f"w1s{f}")
                                nc.sync.dma_start(
                                    wt[:],
                                    dram["w1t"].ap()[f * PC:(f + 1) * PC, :])
                            ps = psp.tile([PC, NF], FP32, tag="fc1",
                                          bufs=3, name=f"h{f}")
                            for kc in range(KC):
                                nc.tensor.matmul(
                                    ps[:],
                                    wt[:, kc * PC:(kc + 1) * PC],
                                    n3b[kc][:], start=(kc == 0),
                                    stop=(kc == KC - 1))
                            nc.scalar.activation(
                                hT[f][:], ps[:], AF.Gelu,
                                bias=b1T[:, f:f + 1])
                        for d in range(8):
                            w2 = s3.tile([PC, F], BF16, tag="w2t",
                                         bufs=2, name=f"w2t{d}")
                            nc.sync.dma_start(
                                w2[:],
                                dram["w2t"].ap()[d * PC:(d + 1) * PC, :])
                            ps = psp.tile([PC, NF], FP32, tag="fc2",
                                          bufs=3, name=f"yp{d}")
                            for f in range(FC):
                                nc.tensor.matmul(
                                    ps[:],
                                    w2[:, f * PC:(f + 1) * PC],
                                    hT[f][:], start=(f == 0),
                                    stop=(f == FC - 1))
                            yt = s3.tile([PC, NF], FP32, tag="yT",
                                         bufs=4, name=f"yT{d}")
                            nc.vector.scalar_tensor_tensor(
                                yt[:], ps[:], b2T[:, d:d + 1],
                                n3T[d][:], op0=ALU.add, op1=ALU.add)
                            nc.sync.dma_start(
                                y_out.ap()[d * PC:(d + 1) * PC, :], yt[:])


def _get_nc():
    if "nc" not in _CACHE:
        _CACHE["nc"] = _build()
    return _CACHE["nc"]


def _prep_shared(inputs):
    """Host-side weight rearrangement shared across cores."""
    Wq = _round_fp32r(inputs["Wq"])
    Wkv = _round_fp32r(inputs["Wkv"])
    Wqkv = _round_fp32r(inputs["Wqkv"])
    W1 = _round_fp32r(inputs["W1"])
    W2 = _round_fp32r(inputs["W2"])
    bq = _round_fp32r(inputs["bq"])
    bkv = _round_fp32r(inputs["bkv"])
    bqkv = _round_fp32r(inputs["bqkv"])

    import ml_dtypes
    BF = ml_dtypes.bfloat16

    def qk_tiles(Wslice):
        # [1024, 512] -> [4*128, 8*128] with row m*128+p, col kc*128+c
        a = Wslice.reshape(KC, PC, 4, PC)           # (kc, p, m, c)
        return np.ascontiguousarray(
            a.transpose(2, 1, 0, 3).reshape(4 * PC, D).astype(BF))

    sh = {}
    for hh in range(2):
        lo = NF * hh
        sh[hh] = {
            "wqt": qk_tiles(Wq[:, lo:lo + NF]),
            "wkt": qk_tiles(Wkv[:, lo:lo + NF]),
            "wq2t": qk_tiles(Wqkv[:, lo:lo + NF]),
            "wk2t": qk_tiles(Wqkv[:, D + lo:D + lo + NF]),
            "wv": np.ascontiguousarray(Wkv[:, D + lo:D + lo + NF].astype(BF)),
            "wv2": np.ascontiguousarray(Wqkv[:, 2 * D + lo:2 * D + lo + NF].astype(BF)),
            "bqc": np.ascontiguousarray(bq[lo:lo + NF].reshape(4, PC).T),
            "bkc": np.ascontiguousarray(bkv[lo:lo + NF].reshape(4, PC).T),
            "bq2c": np.ascontiguousarray(bqkv[lo:lo + NF].reshape(4, PC).T),
            "bk2c": np.ascontiguousarray(
                bqkv[D + lo:D + lo + NF].reshape(4, PC).T),
            "bv": np.ascontiguousarray(bkv[D + lo:D + lo + NF]),
            "bv2": np.ascontiguousarray(bqkv[2 * D + lo:2 * D + lo + NF]),
        }
    # w1t: row f*128+p, col kc*128+c = W1[kc*128+p, f*128+c]
    w1t = np.ascontiguousarray(
        W1.reshape(KC, PC, FC, PC).transpose(2, 1, 0, 3).reshape(F, D)
        .astype(BF))
    # w2t: row d*128+p, col fc*128+c = W2[fc*128+p, d*128+c]
    w2t = np.ascontiguousarray(
        W2.reshape(FC, PC, 8, PC).transpose(2, 1, 0, 3).reshape(D, F)
        .astype(BF))
    gamma = np.ascontiguousarray(np.asarray(inputs["gamma"], np.float32))
    beta = np.ascontiguousarray(np.asarray(inputs["beta"], np.float32))
    b1 = np.ascontiguousarray(np.asarray(inputs["b1"], np.float32))
    b2 = np.ascontiguousarray(np.asarray(inputs["b2"], np.float32))
    common = {
        "w1t": w1t, "w2t": w2t,
        "gammac": np.ascontiguousarray(gamma.reshape(8, PC).T),
        "betac": np.ascontiguousarray(beta.reshape(8, PC).T),
        "b1c": np.ascontiguousarray(b1.reshape(FC, PC).T),
        "b2c": np.ascontiguousarray(b2.reshape(8, PC).T),
    }
    return sh, common


def kernel(**inputs):
    x1 = np.asarray(inputs["x1"], np.float32)
    x2 = np.asarray(inputs["x2"], np.float32)
    sh, common = _prep_shared(inputs)
    import ml_dtypes
    BF = ml_dtypes.bfloat16
    x1t = [np.ascontiguousarray(x1[b].T.astype(BF)) for b in range(B)]
    x2t = [np.ascontiguousarray(x2[b].T.astype(BF)) for b in range(B)]
    x2o = [_round_fp32r(np.ascontiguousarray(x2[b].T)) for b in range(B)]

    nc = _get_nc()
    in_maps = []
    for core in range(8):
        b, hh = core // 2, core % 2
        lo = NF * hh
        m = {"x1t": x1t[b], "x2t": x2t[b],
             "x2o": np.ascontiguousarray(x2o[b][:, lo:lo + NF])}
        m.update(sh[hh])
        m.update(common)
        in_maps.append(m)
    res = run_bass_kernel_spmd(nc, in_maps, core_ids=list(range(8)))
    _CACHE["last_results"] = res
    out = np.zeros((B, N, D), np.float32)
    for core in range(8):
        b, hh = core // 2, core % 2
        out[b, NF * hh:NF * hh + NF, :] = res.results[core]["y"].T
    return out
